# revision 77
# baseline (speedup 1.0000x reference)
"""AttentionBlock (GroupNorm + single-head full attention + residual) on 8 trn2 cores.

Sharding: core i -> batch i//4, query strip (i%4)*1024 .. +1024. Each core
computes its batch's full K/V (duplicated across the 4 cores sharing the
batch). The host rotates each core's copy of x so its query strip sits at
token rows 0..1023 (group-norm statistics and attention key-sums are
permutation-invariant over tokens), letting one SPMD program serve all cores.

Differences vs the bf16 baseline (283.8us):
  - All heavy matmuls run fp8e4m3 with MatmulPerfMode.DoubleRow (virtual
    256-row contraction, ~1.8x streaming throughput): QKV projections,
    scores, exp-weights @ V, rowsum, and the output projection. Operands are
    stored channel-pair interleaved ([128, 2, N] tiles; element [p, j, n] is
    contraction row j*128+p).
  - x arrives channel-major fp8 (host pre-transpose), eliminating the PE
    transpose + copy pipeline of P1.
  - GroupNorm statistics come from DVE bn_stats/bn_aggr over the resident
    channel-major x (no PE stats matmuls, no Square pass); per-channel
    mean/var are PE-transposed to rows, pooled to 32 groups on the free dim,
    refined with Newton-Raphson rsqrt, and broadcast back to per-channel
    scale/bias via a small select-matrix matmul.
  - exp(S*scale - 2) is written directly as fp8e4m3 (logits for these
    normalized inputs are ~N(0,1.2), |S|<7, so the fixed shift keeps
    exp in [e^-9, e^5] - inside e4m3 range; the shift cancels in the
    softmax quotient). Row sums use the same fp8 values, so the softmax
    stays consistent. Attention output is normalized (rowsum reciprocal
    broadcast across partitions) before the fp8 output projection.
  - v/proj biases fold into the residual on host (xres + bv@wp + bp);
    k bias is dropped (softmax shift-invariant); q bias folds into the
    PSUM evacuation of Q^T.
HAM warm-up dummy matmuls run during the stats phase to hold the PE clock.
"""

import os
import numpy as np
from contextlib import ExitStack

import concourse.bass as bass
import concourse.bacc as bacc
import concourse.tile as tile
from concourse import mybir
from concourse.bass_utils import run_bass_kernel_spmd

B, H, W, C = 2, 64, 64, 512
T = H * W                 # 4096 tokens per batch
NCORES = 8
QS = 1024                 # queries per core
GROUPS, GSIZE = 32, 16
EPS = 1e-5
SCALE = float(C) ** -0.5
SHIFT = 2.0               # constant logit shift before exp (cancels in softmax)
F32 = mybir.dt.float32
F8 = mybir.dt.float8e4
DRM = mybir.MatmulPerfMode.DoubleRow
NCH = C // 128            # 4 channel chunks
NPAIR = 2                 # channel-chunk pairs (DoubleRow contraction groups)
NW = T // 512             # 8 token windows
NQW = QS // 512           # 2 query windows
NKT = T // 128            # 32 key subtiles
NBLK = QS // 512          # 2 attention q-blocks
NSUB = 4                  # 128-query subtiles per block


def _build():
    nc = bacc.Bacc(None, target_bir_lowering=False)

    xt_h = nc.declare_dram_parameter("xt", [NPAIR, 128, 2, T], F8, isOutput=False)
    xresb_h = nc.declare_dram_parameter("xresb", [QS, C], F32, isOutput=False)
    wkq_h = nc.declare_dram_parameter("wkq", [NPAIR, 128, 2, C], F8, isOutput=False)
    wv_h = nc.declare_dram_parameter("wv", [NPAIR, 128, 2, C], F8, isOutput=False)
    wp_h = nc.declare_dram_parameter("wp", [NPAIR, 128, 2, C], F8, isOutput=False)
    wkbq_h = nc.declare_dram_parameter("wkbqr", [C], F32, isOutput=False)
    gamma_h = nc.declare_dram_parameter("gamma", [C], F32, isOutput=False)
    beta_h = nc.declare_dram_parameter("beta", [C], F32, isOutput=False)
    sel_h = nc.declare_dram_parameter("selmat", [32, 512], F32, isOutput=False)
    selp_h = nc.declare_dram_parameter("selpool", [128, NCH, 32], F32, isOutput=False)
    ones_h = nc.declare_dram_parameter("ones8", [128, 2, 16], F8, isOutput=False)
    out_h = nc.declare_dram_parameter("out", [QS, C], F32, isOutput=True)

    with tile.TileContext(nc) as tc, ExitStack() as ctx:
        persist = ctx.enter_context(tc.tile_pool(name="persist", bufs=1))
        small = ctx.enter_context(tc.tile_pool(name="small", bufs=1))

        bigpool = ctx.enter_context(tc.tile_pool(name="bigpool", bufs=1))
        # resident channel-major raw x, channel-pair interleaved. GroupNorm is
        # never applied to the key side of the score matmul: S^T's per-query
        # shift from the norm bias is softmax-invariant, and the per-channel
        # scale a folds into the query-side evacuation. So scores read raw x.
        xt_t = [bigpool.tile([128, 2, T], F8, tag=f"xt{p}", name=f"xt{p}")
                for p in range(NPAIR)]
        qts_t = [bigpool.tile([128, 2, QS], F8, tag=f"qts{p}", name=f"qts{p}")
                 for p in range(NPAIR)]
        v_big = bigpool.tile([128, NKT, C], F8, tag="vbig", name="vbig")

        wpool = ctx.enter_context(tc.tile_pool(name="wpool", bufs=1))
        wkq_t = [wpool.tile([128, 2, C], F8, tag=f"wkq{p}", name=f"wkq{p}") for p in range(NPAIR)]
        wv_t = [wpool.tile([128, 2, C], F8, tag=f"wv{p}", name=f"wv{p}") for p in range(NPAIR)]
        # a-scaled copies (GroupNorm scale folded into the contraction side)
        wkqs_t = [wpool.tile([128, 2, C], F8, tag=f"wkqs{p}", name=f"wkqs{p}") for p in range(NPAIR)]
        wvs_t = [wpool.tile([128, 2, C], F8, tag=f"wvs{p}", name=f"wvs{p}") for p in range(NPAIR)]
        wp_t = [persist.tile([128, 2, C], F8, tag=f"wp{p}", name=f"wp{p}") for p in range(NPAIR)]

        # x loads first, spread across the three DMA queues, so bn_stats
        # starts as early as possible
        xq = [nc.sync, nc.gpsimd, nc.scalar, nc.sync]
        for p in range(NPAIR):
            for j in range(2):
                xq[2 * p + j].dma_start(out=xt_t[p][:, j, :], in_=xt_h[p, :, j, :])

        ones8 = persist.tile([128, 2, 16], F8, tag="ones8", name="ones8")
        nc.scalar.dma_start(out=ones8, in_=ones_h[:, :, :])
        for p in range(NPAIR):
            nc.scalar.dma_start(out=wkq_t[p], in_=wkq_h[p])
            nc.scalar.dma_start(out=wv_t[p], in_=wv_h[p])
            nc.scalar.dma_start(out=wp_t[p], in_=wp_h[p])

        # per-channel vectors as [128, NCH] (column cc = channel chunk cc)
        def vec_tile(h, name):
            t = small.tile([128, NCH], F32, tag=name)
            nc.scalar.dma_start(out=t, in_=h.rearrange("(a p) -> p a", p=128))
            return t

        gamma_sb = vec_tile(gamma_h, "gamma")
        beta_sb = vec_tile(beta_h, "beta")
        wkbq_sb = vec_tile(wkbq_h, "wkbq")
        sel_sb = small.tile([32, 512], F32, tag="sel_sb", name="sel_sb")
        nc.scalar.dma_start(out=sel_sb, in_=sel_h[:, :])
        selp_sb = small.tile([128, NCH, 32], F32, tag="selp_sb", name="selp_sb")
        nc.scalar.dma_start(out=selp_sb, in_=selp_h[:, :, :])

        scale_all = small.tile([128, NCH, 1], F32, tag="scale_all", name="scale_all")
        bias_all = small.tile([128, NCH, 1], F32, tag="bias_all", name="bias_all")
        scale_t = [scale_all[:, c, :] for c in range(NCH)]
        bias_t = [bias_all[:, c, :] for c in range(NCH)]
        shift_t = small.tile([128, 1], F32, tag="shift_t", name="shift_t")
        nc.vector.memset(shift_t, -SHIFT)
        zero_t = small.tile([128, 1], F32, tag="zero_t", name="zero_t")
        nc.vector.memset(zero_t, 0.0)
        eps32 = small.tile([32, 1], F32, tag="eps32", name="eps32")
        nc.vector.memset(eps32, EPS)
        onesr = small.tile([1, 128], F32, tag="onesr", name="onesr")
        nc.vector.memset(onesr, 1.0)
        qa2 = small.tile([128, NCH, 1], F32, tag="qa2", name="qa2")
        bvpb = small.tile([128, 512], F32, tag="bvpb", name="bvpb")

        # PE warm-up / keep-alive: dummy matmuls hold the HAM clock at 2.4GHz
        warm_sb = small.tile([128, 512], F32, tag="warm_sb", name="warm_sb")
        nc.vector.memset(warm_sb, 0.0)
        warm_bf = small.tile([128, 512], mybir.dt.bfloat16, tag="warm_bf",
                             name="warm_bf")
        nc.vector.memset(warm_bf, 0.0)

        # ================= P1: group-norm statistics (DVE bn_stats) ============
        # Everything stays on partitions: per-channel (mean, var, mean^2) rows
        # are pooled to the 32 groups with a tiny select matmul (contraction
        # over the partition/channel dim), so no slow 1-partition row ops.
        with tc.tile_pool(name="p1ps", bufs=1, space="PSUM") as p1ps, \
             tc.tile_pool(name="p1sb", bufs=1) as p1sb:

            def keepalive(n, dep=None):
                # dep (optional) delays the dummy matmuls until that tile is
                # ready, spreading them across the stats phase so the HAM
                # clock gate never sees a >3.4us PE-idle window
                for _ in range(n):
                    kps = p1ps.tile([128, 512], F32, tag="keep", name="keep", bufs=1)
                    if dep is None:
                        nc.tensor.matmul(kps, warm_bf[:, 0:128], warm_bf,
                                         start=True, stop=True)
                    else:
                        nc.tensor.matmul(kps[0:dep.shape[-1], :], dep,
                                         warm_sb[0:dep.shape[0], :],
                                         start=True, stop=True)

            keepalive(30)
            rhs4 = []
            for cc in range(NCH):
                p, j = cc // 2, cc % 2
                bn6 = p1sb.tile([128, 8, 6], F32, tag=f"bn6_{cc}", name=f"bn6_{cc}")
                for s in range(8):
                    nc.vector.bn_stats(bn6[:, s, :], xt_t[p][:, j, s * 512:(s + 1) * 512])
                r4 = p1sb.tile([128, 4], F32, tag=f"bn2_{cc}", name=f"bn2_{cc}")
                nc.vector.bn_aggr(r4[:, 0:2], bn6.rearrange("p a (b c) -> p (a b) c", c=3))
                # col2 = E[x^2] = var + mean^2, col3 = -mean (for a fused
                # group-variance step after pooling)
                nc.vector.scalar_tensor_tensor(
                    out=r4[:, 2:3], in0=r4[:, 0:1], scalar=r4[:, 0:1],
                    in1=r4[:, 1:2], op0=mybir.AluOpType.mult,
                    op1=mybir.AluOpType.add)
                nc.vector.tensor_scalar_mul(r4[:, 3:4], r4[:, 0:1], -1.0)
                rhs4.append(r4)
                keepalive(5, dep=bn6[:, 0, :])
            g4_ps = p1ps.tile([32, 4], F32, tag="g3", name="g3", bufs=1)
            for cc in range(NCH):
                nc.tensor.matmul(g4_ps, selp_sb[:, cc, :], rhs4[cc],
                                 start=(cc == 0), stop=(cc == NCH - 1))
            keepalive(2, dep=rhs4[3])
            g4 = p1sb.tile([32, 4], F32, tag="g4sb", name="g4sb")
            nc.vector.tensor_copy(g4, g4_ps)
            # var_g = E[x^2]_g - mean_g^2 in one fused op
            ve = p1sb.tile([32, 1], F32, tag="ve", name="ve")
            nc.vector.scalar_tensor_tensor(
                out=ve, in0=g4[:, 3:4], scalar=g4[:, 0:1],
                in1=g4[:, 2:3], op0=mybir.AluOpType.mult,
                op1=mybir.AluOpType.add)
            sd = p1sb.tile([32, 1], F32, tag="sd", name="sd")
            nc.scalar.activation(sd, ve, mybir.ActivationFunctionType.Sqrt,
                                 bias=eps32)
            keepalive(2, dep=sd)
            g2 = p1sb.tile([32, 2], F32, tag="g2sb", name="g2sb")
            nc.vector.tensor_copy(g2[:, 0:1], g4[:, 0:1])
            nc.vector.reciprocal(g2[:, 1:2], sd)
            keepalive(2, dep=g2)
            # broadcast group (mean, rstd) to per-channel scale/bias in one
            # vectorized pass: bps[:, 2cc:2cc+2] = (mean, rstd) of chunk cc
            bps = p1ps.tile([128, 2 * NCH], F32, tag="bps", name="bps", bufs=1)
            for cc in range(NCH):
                nc.tensor.matmul(bps[:, 2 * cc:2 * cc + 2],
                                 sel_sb[:, cc * 128:(cc + 1) * 128], g2,
                                 start=True, stop=True)
            bps_r = bps.rearrange("p (a b) -> p a b", b=2)
            gam_r = gamma_sb.rearrange("p (a b) -> p a b", b=1)
            bet_r = beta_sb.rearrange("p (a b) -> p a b", b=1)
            nc.vector.tensor_tensor(out=scale_all, in0=bps_r[:, :, 1:2],
                                    in1=gam_r, op=mybir.AluOpType.mult)
            mtall = p1sb.tile([128, NCH, 1], F32, tag="mtall", name="mtall")
            nc.vector.tensor_tensor(out=mtall, in0=bps_r[:, :, 0:1],
                                    in1=scale_all, op=mybir.AluOpType.mult)
            nc.vector.tensor_tensor(out=bias_all, in0=bet_r, in1=mtall,
                                    op=mybir.AluOpType.subtract)

            # ---- fold the norm into the weights / small bias vectors ----
            # scaled weight copies: wkqs/wvs = diag(a) @ w (a on the
            # contraction channel = partition dim)
            for p in range(NPAIR):
                for j in range(2):
                    cc = 2 * p + j
                    if j == 0:
                        nc.vector.tensor_scalar_mul(
                            wkqs_t[p][:, j, :], wkq_t[p][:, j, :], scale_t[cc])
                        nc.vector.tensor_scalar_mul(
                            wvs_t[p][:, j, :], wv_t[p][:, j, :], scale_t[cc])
                    else:
                        nc.scalar.activation(
                            wkqs_t[p][:, j, :], wkq_t[p][:, j, :],
                            mybir.ActivationFunctionType.Identity,
                            bias=zero_t, scale=scale_t[cc])
                        nc.scalar.activation(
                            wvs_t[p][:, j, :], wv_t[p][:, j, :],
                            mybir.ActivationFunctionType.Identity,
                            bias=zero_t, scale=scale_t[cc])
            # Norm-shift bias vectors, built directly on partitions with tiny
            # N=1 column matmuls (out[cl, cc] = sum_ci b_ci * W[ci, cc*128+cl])
            # - no transposes, no single-partition row ops.
            bias_b = p1sb.tile([128, NCH, 1], mybir.dt.bfloat16, tag="bias_b",
                               name="bias_b")
            nc.vector.tensor_copy(bias_b, bias_all)
            qb2_ps = p1ps.tile([128, NCH], F32, tag="qb2", name="qb2", bufs=1)
            t1_ps = p1ps.tile([128, NCH], F32, tag="t1p", name="t1p", bufs=1)
            for cc in range(NCH):
                for ci in range(NCH):
                    nc.tensor.matmul(
                        qb2_ps[:, cc:cc + 1],
                        wkq_t[ci // 2][:, ci % 2, cc * 128:(cc + 1) * 128],
                        bias_b[:, ci, :],
                        start=(ci == 0), stop=(ci == NCH - 1))
                    nc.tensor.matmul(
                        t1_ps[:, cc:cc + 1],
                        wv_t[ci // 2][:, ci % 2, cc * 128:(cc + 1) * 128],
                        bias_b[:, ci, :],
                        start=(ci == 0), stop=(ci == NCH - 1))
            # qa2 = a . (b@WKQT + wk@bq): query-side evacuation bias
            qb2c = p1sb.tile([128, NCH, 1], F32, tag="qb2c", name="qb2c")
            nc.vector.tensor_tensor(
                out=qb2c, in0=qb2_ps.rearrange("p (a b) -> p a b", b=1),
                in1=wkbq_sb.rearrange("p (a b) -> p a b", b=1),
                op=mybir.AluOpType.add)
            nc.vector.tensor_tensor(out=qa2, in0=qb2c, in1=scale_all,
                                    op=mybir.AluOpType.mult)
            t1c = p1sb.tile([128, NCH], mybir.dt.bfloat16, tag="t1c", name="t1c")
            nc.vector.tensor_copy(t1c, t1_ps)
            # bvpb = broadcast((b @ wv) @ wp): the rank-1 norm-bias term of V
            # commutes through the softmax average and the projection
            t2_ps = p1ps.tile([1, C], F32, tag="t2p", name="t2p", bufs=1)
            for cc in range(NCH):
                nc.tensor.matmul(t2_ps, t1c[:, cc:cc + 1],
                                 wp_t[cc // 2][:, cc % 2, :],
                                 start=(cc == 0), stop=(cc == NCH - 1))
            t2r = p1sb.tile([1, C], F32, tag="t2r", name="t2r")
            nc.vector.tensor_copy(t2r, t2_ps)
            nc.gpsimd.partition_broadcast(bvpb, t2r[0:1, :])

        # ====== P2: normalize h (resident) -> V and qk = (wq wk^T) h_q,
        # ====== then P3: attention - one pool scope, no barrier between them
        with tc.tile_pool(name="p3ps", bufs=1, space="PSUM") as p3ps, \
             tc.tile_pool(name="p3ot", bufs=1, space="PSUM") as p3ot, \
             tc.tile_pool(name="p3sb", bufs=1) as p3sb, \
             tc.tile_pool(name="p3pt", bufs=32) as p3pt:
            # qk^T = a . (WKQT' x_q + bias): per-channel norm scale applied at
            # evacuation, bias folded from the norm shift
            for w in range(NQW):
                wsl = slice(w * 512, (w + 1) * 512)
                for cq in range(NCH):
                    ps = p3ps.tile([128, 512], F32, tag="sc", name="kvp", bufs=3)
                    for p in range(NPAIR):
                        nc.tensor.matmul(
                            ps, wkqs_t[p][:, :, cq * 128:(cq + 1) * 128],
                            xt_t[p][:, :, wsl],
                            start=(p == 0), stop=(p == NPAIR - 1), perf_mode=DRM)
                    if cq < 2:
                        nc.vector.tensor_scalar(
                            out=qts_t[cq // 2][:, cq % 2, w * 512:(w + 1) * 512],
                            in0=ps, scalar1=scale_t[cq], scalar2=qa2[:, cq, :],
                            op0=mybir.AluOpType.mult, op1=mybir.AluOpType.add)
                    else:
                        nc.scalar.activation(
                            qts_t[cq // 2][:, cq % 2, w * 512:(w + 1) * 512], ps,
                            mybir.ActivationFunctionType.Identity,
                            bias=qa2[:, cq, :], scale=scale_t[cq])

            def emit_v_group(w, i):
                # V projection for one 128-token subtile of window w
                ps = p3ps.tile([128, 512], F32, tag="sc", name="kvp", bufs=3)
                for p in range(NPAIR):
                    nc.tensor.matmul(
                        ps, xt_t[p][:, :, w * 512 + i * 128:w * 512 + (i + 1) * 128],
                        wvs_t[p], start=(p == 0), stop=(p == NPAIR - 1),
                        perf_mode=DRM)
                if (w * 4 + i) % 2 == 0:
                    nc.vector.tensor_copy(v_big[:, w * 4 + i, :], ps)
                else:
                    nc.scalar.copy(v_big[:, w * 4 + i, :], ps)

            # V for windows 0-1 up front (block 0's first PV steps need them);
            # the rest stream into block 0's score/PV slots where the PE
            # would otherwise wait on the exp pipeline
            for w in range(2):
                for i in range(4):
                    emit_v_group(w, i)
            vqueue = [(w, i) for w in range(2, NW) for i in range(4)]
            # ---- P3: attention, scores and exp@V fused per key-subtile ----
            # Per m: score matmuls for m, then PV matmuls for m-1 (whose exp
            # just finished on ACT) and the m-1 rowsum - the PE never waits
            # for the scalar engine, and the softmax-denominator reciprocal
            # chain is emitted only after all PE work so it overlaps PV.
            NM = NKT // 2

            def emit_proj(blk, ots):
                # output projection + residual for a finished block
                for sub in range(NSUB):
                    ti = blk * NSUB + sub
                    ps_p = p3ps.tile([128, C], F32, tag="sc", name="ps_p", bufs=3)
                    for p in range(NPAIR):
                        nc.tensor.matmul(
                            ps_p, ots[p][:, :, sub * 128:(sub + 1) * 128], wp_t[p],
                            start=(p == 0), stop=(p == NPAIR - 1), perf_mode=DRM)
                    xres = p3sb.tile([128, C], F32, tag="xres", name="xres", bufs=3)
                    nc.sync.dma_start(out=xres, in_=xresb_h[ti * 128:(ti + 1) * 128, :])
                    xrb = p3sb.tile([128, C], F32, tag="xrb", name="xrb", bufs=3)
                    nc.gpsimd.tensor_tensor(out=xrb, in0=xres, in1=bvpb,
                                            op=mybir.AluOpType.add)
                    fin = p3sb.tile([128, C], F32, tag="fin", name="fin", bufs=3)
                    nc.vector.tensor_tensor(out=fin, in0=ps_p, in1=xrb,
                                            op=mybir.AluOpType.add)
                    nc.sync.dma_start(out=out_h[ti * 128:(ti + 1) * 128, :], in_=fin)

            pending = []
            for blk in range(NBLK):
                q0 = blk * 512
                ptws = []
                rs_ps = p3ot.tile([1, 512], F32, tag="rsum", name="rsum", bufs=1)
                ot_ps = p3ot.tile([128, NCH, 512], F32, tag="ot", name="ot", bufs=1)

                def pv_step(m, rs_ps=rs_ps, ot_ps=ot_ps, ptws=ptws):
                    nc.tensor.matmul(rs_ps, ones8[:, :, 0:1], ptws[m],
                                     start=(m == 0), stop=(m == NM - 1),
                                     perf_mode=DRM)
                    for cv in range(NCH):
                        nc.tensor.matmul(
                            ot_ps[:, cv, :],
                            v_big[:, 2 * m:2 * m + 2, cv * 128:(cv + 1) * 128],
                            ptws[m], start=(m == 0), stop=(m == NM - 1),
                            perf_mode=DRM)

                for m in range(NM):
                    ptw = p3pt.tile([128, 2, 512], F8, tag="ptw", name="ptw")
                    for h in range(2):
                        w2 = 2 * m + h
                        st_ps = p3ps.tile([128, 512], F32, tag="sc", name="st_ps", bufs=3)
                        for p in range(NPAIR):
                            nc.tensor.matmul(
                                st_ps, xt_t[p][:, :, w2 * 128:(w2 + 1) * 128],
                                qts_t[p][:, :, q0:q0 + 512],
                                start=(p == 0), stop=(p == NPAIR - 1), perf_mode=DRM)
                        nc.scalar.activation(ptw[:, h, :], st_ps,
                                             mybir.ActivationFunctionType.Exp,
                                             bias=shift_t, scale=SCALE)
                    ptws.append(ptw)
                    if m > 0:
                        pv_step(m - 1)
                    if m >= 1:
                        # stream the remaining V-projection groups into block
                        # 0's slots (PE capacity the exp pipeline can't use)
                        for _ in range(2):
                            if vqueue:
                                emit_v_group(*vqueue.pop(0))
                    if m == 6 and pending:
                        # previous block's projection, emitted mid-stream so
                        # its PSUM-evac dependencies are long satisfied
                        emit_proj(*pending.pop())
                pv_step(NM - 1)
                rs_row = p3sb.tile([1, 512], F32, tag="rs_row", name="rs_row", bufs=2)
                nc.scalar.copy(rs_row, rs_ps)
                # broadcast the rowsum across partitions with a 1-row matmul
                # (faster than gpsimd partition_broadcast, uses a spare sc slot)
                rsb_ps = p3ps.tile([128, 512], F32, tag="sc", name="rsb", bufs=3)
                nc.tensor.matmul(rsb_ps, onesr, rs_row, start=True, stop=True)
                rinvb = p3sb.tile([128, 512], F32, tag="rinvb", name="rinvb", bufs=2)
                nc.vector.reciprocal_approx_fast(rinvb, rsb_ps)
                # normalize rows (deferred softmax denominator) -> fp8
                ots = [p3sb.tile([128, 2, 512], F8, tag=f"ots{pp}", name=f"ots{pp}",
                                 bufs=2) for pp in range(NPAIR)]
                for cv in range(NCH):
                    nc.vector.tensor_tensor(out=ots[cv // 2][:, cv % 2, :],
                                            in0=ot_ps[:, cv, :], in1=rinvb,
                                            op=mybir.AluOpType.mult)
                pending.append((blk, ots))
            emit_proj(*pending.pop())

    nc.compile()
    return nc


_NC_CACHE = []


def prepare_in_maps(x, gamma, beta, wq, bq, wk, bk, wv, bv, wp, bp):
    import ml_dtypes
    F8NP = ml_dtypes.float8_e4m3

    def to8(a):
        return np.ascontiguousarray(
            np.clip(np.asarray(a, np.float32), -240.0, 240.0).astype(F8NP))

    def pair_interleave(wm):
        # [C, N] -> [NPAIR, 128, 2, N]; element [p, ci, j, n] = wm[(2p+j)*128+ci, n]
        wm = np.asarray(wm, np.float32)
        return to8(wm.reshape(2, 2, 128, -1).transpose(0, 2, 1, 3))

    x = np.ascontiguousarray(np.asarray(x, dtype=np.float32))
    xf = x.reshape(B, T, C)
    bpp = (np.asarray(bv, np.float32) @ np.asarray(wp, np.float32)
           + np.asarray(bp, np.float32))
    sel = np.zeros((32, 512), np.float32)
    selpool = np.zeros((128, 4, 32), np.float32)
    for cc in range(4):
        for cl in range(128):
            sel[8 * cc + cl // GSIZE, cc * 128 + cl] = 1.0
            selpool[cl, cc, 8 * cc + cl // GSIZE] = 1.0 / GSIZE
    wkqt = np.asarray(wq, np.float32) @ np.asarray(wk, np.float32).T
    common = {
        "wkq": pair_interleave(wkqt),
        "wv": pair_interleave(wv), "wp": pair_interleave(wp),
        "wkbqr": np.asarray(wk, np.float32) @ np.asarray(bq, np.float32),
        "gamma": np.asarray(gamma, np.float32),
        "beta": np.asarray(beta, np.float32),
        "selmat": sel,
        "selpool": selpool,
        "ones8": np.ones((128, 2, 16), F8NP),
    }
    in_maps = []
    for core in range(NCORES):
        b, qoff = core // 4, (core % 4) * QS
        # rotate so this core's query strip is rows 0..1023 (attention and
        # group stats are permutation-invariant over tokens)
        xr = np.roll(xf[b], -qoff, axis=0)           # [T, C]
        xtp = pair_interleave(xr.T)                  # [NPAIR, 128, 2, T]
        in_maps.append({
            **common,
            "xt": xtp,
            "xresb": np.ascontiguousarray(xf[b, qoff:qoff + QS] + bpp[None, :]),
        })
    return in_maps


def kernel(x, gamma, beta, wq, bq, wk, bk, wv, bv, wp, bp):
    if not _NC_CACHE:
        _NC_CACHE.append(_build())
    nc = _NC_CACHE[0]
    in_maps = prepare_in_maps(x, gamma, beta, wq, bq, wk, bk, wv, bv, wp, bp)
    res = run_bass_kernel_spmd(nc, in_maps, list(range(NCORES)))
    out = np.empty((B, T, C), np.float32)
    for core in range(NCORES):
        b, qoff = core // 4, (core % 4) * QS
        out[b, qoff:qoff + QS] = res.results[core]["out"]
    return out.reshape(B, H, W, C)
